# revision 1
# baseline (speedup 1.0000x reference)
"""MoE (shared expert + 8 routed experts, top-2) on 8 TRN2 NeuronCores.

Strategy: data-parallel over tokens. Each core takes 1024 of the 8192 tokens
and computes the router (softmax + top-2 combine weights) and all 9 dense
SwiGLU expert MLPs (shared + 8 routed; routed scaled by per-token combine
weight, zero for non-routed tokens — mathematically identical to sparse
dispatch). Matmuls run in fp32r (full PE speed, ~1e-3 absolute accuracy).
Outputs concatenate across cores — no collectives needed.

Self-contained: hardcodes all shapes; relies only on the ambient concourse
environment (axon-tunneled TRN2 cores).
"""
import numpy as np

import bass_rust
import concourse.bass as bass
import concourse.mybir as mybir
import concourse.tile as tile
from concourse.bass_utils import run_bass_kernel_spmd

D = 1024          # d_model
H = 2048          # d_hidden per expert
E = 8             # routed experts
NCORES = 8
TPC = 1024        # tokens per core (8192 total / 8)
DK = D // 128     # 8 contraction tiles over d_model
HK = H // 128     # 16 contraction tiles over d_hidden
TT = TPC // 128   # 8 token tiles per core
NEXP = E + 1      # shared expert handled as expert 0 with combine weight 1

F32 = mybir.dt.float32
F32R = mybir.dt.float32r
AX = mybir.AxisListType.X
ALU = mybir.AluOpType
AF = mybir.ActivationFunctionType

_waitfix_ctr = [0]


def _normalize_waits(nc, max_waits=1):
    """This environment's walrus codegen accepts only one sync-wait slot per
    instruction; hoist extras onto standalone event-sem instructions."""
    for f in nc.m.functions:
        for blk in f.blocks:
            insts = list(blk.instructions)
            out = []
            changed = False
            for inst in insts:
                si = inst.sync_info
                waits = list(si.on_wait) if (si is not None and si.on_wait) else []
                if len(waits) > max_waits:
                    extra, keep = waits[:-max_waits], waits[-max_waits:]
                    for w in extra:
                        _waitfix_ctr[0] += 1
                        ev = mybir.InstEventSemaphore(
                            name=f"waitfix_{_waitfix_ctr[0]}", ins=[], outs=[]
                        )
                        ev.engine = inst.engine
                        ev.sync_info = bass_rust.SyncInfo(on_wait=[w], on_update=[])
                        out.append(ev)
                    si.on_wait = keep
                    changed = True
                out.append(inst)
            if changed:
                blk.instructions.clear()
                blk.instructions.extend(out)


def build_nc():
    nc = bass.Bass()

    # fp32r dram views of the (raw fp32) inputs; PE reads them at full rate.
    xT = nc.dram_tensor("xT", [D, TPC], F32R, kind="ExternalInput")
    router_w = nc.dram_tensor("router_w", [D, E], F32R, kind="ExternalInput")
    # gate/up: [9, D, H] (expert 0 = shared); down: [9, H, D]
    gate_all = nc.dram_tensor("gate_all", [NEXP, D, H], F32R, kind="ExternalInput")
    up_all = nc.dram_tensor("up_all", [NEXP, D, H], F32R, kind="ExternalInput")
    down_all = nc.dram_tensor("down_all", [NEXP, H, D], F32R, kind="ExternalInput")
    out = nc.dram_tensor("out", [TPC, D], F32, kind="ExternalOutput")

    with tile.TileContext(nc) as tc:
        with (
            tc.tile_pool(name="big", bufs=1) as big,       # persistent tiles
            tc.tile_pool(name="wpool", bufs=2) as wpool,   # streamed weight tiles
            tc.tile_pool(name="small", bufs=3) as small,   # elementwise temps
            tc.tile_pool(name="ps", bufs=2, space="PSUM") as ps,
        ):
            # ---- load x^T resident: [128, DK, TPC] (32KB/partition) ----
            xt = big.tile([128, DK, TPC], F32R)
            nc.sync.dma_start(xt, xT[:, :].rearrange("(dk p) t -> p dk t", p=128))

            # ---- router: probs, top-2 combine weights ----
            rw = big.tile([128, DK, E], F32R)
            nc.sync.dma_start(rw, router_w[:, :].rearrange("(dk p) e -> p dk e", p=128))

            # combs[:, t, 0] = 1 (shared), combs[:, t, 1+e] = combine weight
            combs = big.tile([128, TT, NEXP], F32)
            for t in range(TT):
                psl = ps.tile([128, 512], F32, tag="l")
                pl = psl[:, :E]
                for dk in range(DK):
                    nc.tensor.matmul(
                        pl,
                        xt[:, dk, t * 128:(t + 1) * 128],
                        rw[:, dk],
                        start=(dk == 0),
                        stop=(dk == DK - 1),
                    )
                mx = small.tile([128, 1], F32, tag="mx")
                nc.vector.reduce_max(mx, pl, axis=AX)
                negmx = small.tile([128, 1], F32, tag="negmx")
                nc.vector.tensor_scalar_mul(negmx, mx, -1.0)
                ex = small.tile([128, E], F32, tag="ex")
                nc.scalar.activation(ex, pl, AF.Exp, bias=negmx, scale=1.0)
                sm = small.tile([128, 1], F32, tag="sm")
                nc.vector.reduce_sum(sm, ex, axis=AX)
                rs = small.tile([128, 1], F32, tag="rs")
                nc.vector.reciprocal(rs, sm)
                probs = small.tile([128, E], F32, tag="probs")
                nc.vector.tensor_scalar_mul(probs, ex, rs)
                # top-1
                m1 = small.tile([128, 1], F32, tag="m1")
                nc.vector.reduce_max(m1, probs, axis=AX)
                t1 = small.tile([128, E], F32, tag="t1")
                nc.vector.tensor_scalar(t1, probs, m1, None, ALU.is_ge)
                ptop = small.tile([128, E], F32, tag="ptop")
                nc.vector.tensor_mul(ptop, probs, t1)
                pm = small.tile([128, E], F32, tag="pm")
                nc.vector.tensor_sub(pm, probs, ptop)
                # top-2
                m2 = small.tile([128, 1], F32, tag="m2")
                nc.vector.reduce_max(m2, pm, axis=AX)
                t2 = small.tile([128, E], F32, tag="t2")
                nc.vector.tensor_scalar(t2, pm, m2, None, ALU.is_ge)
                p2 = small.tile([128, E], F32, tag="p2")
                nc.vector.tensor_mul(p2, pm, t2)
                nc.vector.tensor_add(combs[:, t, 1:], ptop, p2)
                nc.vector.memset(combs[:, t, 0:1], 1.0)

            # ---- output accumulator [128, TT, D] f32 (32KB/partition) ----
            out_acc = big.tile([128, TT, D], F32)

            # ---- expert loop: shared (j=0) + routed (j=1..8) ----
            hs = big.tile([128, HK, TPC], F32R)  # silu(G)*U, transposed [h, tok]
            for j in range(NEXP):
                # gate/up phase: 2 h-tiles (256 cols) per weight DMA
                for hb2 in range(HK // 2):
                    wg = wpool.tile([128, DK, 256], F32R, tag="wg")
                    wu = wpool.tile([128, DK, 256], F32R, tag="wu")
                    nc.sync.dma_start(
                        wg,
                        gate_all[j, :, hb2 * 256:(hb2 + 1) * 256].rearrange(
                            "(dk p) h -> p dk h", p=128
                        ),
                    )
                    nc.sync.dma_start(
                        wu,
                        up_all[j, :, hb2 * 256:(hb2 + 1) * 256].rearrange(
                            "(dk p) h -> p dk h", p=128
                        ),
                    )
                    for hh in range(2):
                        hb = hb2 * 2 + hh
                        for th in range(2):
                            tsl = slice(th * 512, (th + 1) * 512)
                            pg = ps.tile([128, 512], F32, tag="g")
                            pu = ps.tile([128, 512], F32, tag="u")
                            for dk in range(DK):
                                nc.tensor.matmul(
                                    pg,
                                    wg[:, dk, hh * 128:(hh + 1) * 128],
                                    xt[:, dk, tsl],
                                    start=(dk == 0),
                                    stop=(dk == DK - 1),
                                )
                            for dk in range(DK):
                                nc.tensor.matmul(
                                    pu,
                                    wu[:, dk, hh * 128:(hh + 1) * 128],
                                    xt[:, dk, tsl],
                                    start=(dk == 0),
                                    stop=(dk == DK - 1),
                                )
                            sg = small.tile([128, 512], F32, tag="sg")
                            nc.scalar.activation(sg, pg, AF.Silu)
                            nc.vector.tensor_mul(hs[:, hb, tsl], sg, pu)

                # down phase: d in quarters of 256
                for dq in range(4):
                    wd = wpool.tile([128, HK, 256], F32R, tag="wd")
                    nc.sync.dma_start(
                        wd,
                        down_all[j, :, dq * 256:(dq + 1) * 256].rearrange(
                            "(hk p) d -> p hk d", p=128
                        ),
                    )
                    for t in range(TT):
                        pz = ps.tile([128, 256], F32, tag="z")
                        for hk in range(HK):
                            nc.tensor.matmul(
                                pz,
                                hs[:, hk, t * 128:(t + 1) * 128],
                                wd[:, hk],
                                start=(hk == 0),
                                stop=(hk == HK - 1),
                            )
                        osl = out_acc[:, t, dq * 256:(dq + 1) * 256]
                        if j == 0:
                            nc.scalar.activation(osl, pz, AF.Copy)
                        else:
                            zt = small.tile([128, 256], F32, tag="zt")
                            nc.scalar.activation(
                                zt, pz, AF.Copy, scale=combs[:, t, j:j + 1]
                            )
                            nc.vector.tensor_add(osl, osl, zt)

            # ---- store ----
            nc.sync.dma_start(
                out[:, :].rearrange("(t p) d -> p t d", p=128), out_acc
            )

    _normalize_waits(nc)
    return nc


_built = {}


def _get_nc():
    if "nc" not in _built:
        _built["nc"] = build_nc()
    return _built["nc"]


def prepare_in_maps(x, router_w, shared_gate, shared_up, shared_down,
                    gate_w, up_w, down_w):
    xf = np.ascontiguousarray(np.asarray(x, np.float32).reshape(-1, D))
    gate_all = np.concatenate(
        [np.asarray(shared_gate, np.float32)[None], np.asarray(gate_w, np.float32)], 0
    )
    up_all = np.concatenate(
        [np.asarray(shared_up, np.float32)[None], np.asarray(up_w, np.float32)], 0
    )
    down_all = np.concatenate(
        [np.asarray(shared_down, np.float32)[None], np.asarray(down_w, np.float32)], 0
    )
    rw = np.ascontiguousarray(np.asarray(router_w, np.float32))
    in_maps = []
    for c in range(NCORES):
        xs = xf[c * TPC:(c + 1) * TPC]
        in_maps.append({
            "xT": np.ascontiguousarray(xs.T),
            "router_w": rw,
            "gate_all": gate_all,
            "up_all": up_all,
            "down_all": down_all,
        })
    return in_maps


def kernel(x, router_w, shared_gate, shared_up, shared_down,
           gate_w, up_w, down_w, top_k):
    assert int(top_k) == 2, "kernel hardcodes top-2 routing"
    nc = _get_nc()
    in_maps = prepare_in_maps(
        x, router_w, shared_gate, shared_up, shared_down, gate_w, up_w, down_w
    )
    res = run_bass_kernel_spmd(nc, in_maps, list(range(NCORES)), trace=False)
    outs = [res.results[c]["out"] for c in range(NCORES)]
    full = np.concatenate(outs, axis=0)
    return full.reshape(np.asarray(x).shape).astype(np.float32)


# revision 2
# speedup vs baseline: 1.0038x; 1.0038x over previous
"""MoE (shared expert + 8 routed experts, top-2) on 8 TRN2 NeuronCores.

Strategy: data-parallel over tokens. Each core takes 1024 of the 8192 tokens
and computes the router (softmax + top-2 combine weights) and all 9 dense
SwiGLU expert MLPs (shared + 8 routed; routed scaled by per-token combine
weight, zero for non-routed tokens — mathematically identical to sparse
dispatch). Matmuls run in fp32r (full PE speed, ~1e-3 absolute accuracy).
Outputs concatenate across cores — no collectives needed.

Self-contained: hardcodes all shapes; relies only on the ambient concourse
environment (axon-tunneled TRN2 cores).
"""
import numpy as np

import bass_rust
import concourse.bass as bass
import concourse.mybir as mybir
import concourse.tile as tile
from concourse.bass_utils import run_bass_kernel_spmd

D = 1024          # d_model
H = 2048          # d_hidden per expert
E = 8             # routed experts
NCORES = 8
TPC = 1024        # tokens per core (8192 total / 8)
DK = D // 128     # 8 contraction tiles over d_model
HK = H // 128     # 16 contraction tiles over d_hidden
TT = TPC // 128   # 8 token tiles per core
NEXP = E + 1      # shared expert handled as expert 0 with combine weight 1

F32 = mybir.dt.float32
F32R = mybir.dt.float32r
AX = mybir.AxisListType.X
ALU = mybir.AluOpType
AF = mybir.ActivationFunctionType

_waitfix_ctr = [0]


def _normalize_waits(nc, max_waits=1):
    """This environment's walrus codegen accepts only one sync-wait slot per
    instruction; hoist extras onto standalone event-sem instructions."""
    for f in nc.m.functions:
        for blk in f.blocks:
            insts = list(blk.instructions)
            out = []
            changed = False
            for inst in insts:
                si = inst.sync_info
                waits = list(si.on_wait) if (si is not None and si.on_wait) else []
                if len(waits) > max_waits:
                    extra, keep = waits[:-max_waits], waits[-max_waits:]
                    for w in extra:
                        _waitfix_ctr[0] += 1
                        ev = mybir.InstEventSemaphore(
                            name=f"waitfix_{_waitfix_ctr[0]}", ins=[], outs=[]
                        )
                        ev.engine = inst.engine
                        ev.sync_info = bass_rust.SyncInfo(on_wait=[w], on_update=[])
                        out.append(ev)
                    si.on_wait = keep
                    changed = True
                out.append(inst)
            if changed:
                blk.instructions.clear()
                blk.instructions.extend(out)


def build_nc():
    nc = bass.Bass()

    # fp32r dram views of the (raw fp32) inputs; PE reads them at full rate.
    xT = nc.dram_tensor("xT", [D, TPC], F32R, kind="ExternalInput")
    router_w = nc.dram_tensor("router_w", [D, E], F32R, kind="ExternalInput")
    # gate/up: [9, D, H] (expert 0 = shared); down: [9, H, D]
    gate_all = nc.dram_tensor("gate_all", [NEXP, D, H], F32R, kind="ExternalInput")
    up_all = nc.dram_tensor("up_all", [NEXP, D, H], F32R, kind="ExternalInput")
    down_all = nc.dram_tensor("down_all", [NEXP, H, D], F32R, kind="ExternalInput")
    out = nc.dram_tensor("out", [TPC, D], F32, kind="ExternalOutput")

    with tile.TileContext(nc) as tc:
        with (
            tc.tile_pool(name="big", bufs=1) as big,       # persistent tiles
            tc.tile_pool(name="wpool", bufs=2) as wpool,   # streamed weight tiles
            tc.tile_pool(name="small", bufs=3) as small,   # elementwise temps
            tc.tile_pool(name="ps", bufs=2, space="PSUM") as ps,
        ):
            # ---- load x^T resident: [128, DK, TPC] (32KB/partition) ----
            xt = big.tile([128, DK, TPC], F32R)
            nc.sync.dma_start(xt, xT[:, :].rearrange("(dk p) t -> p dk t", p=128))

            # ---- router: probs, top-2 combine weights ----
            rw = big.tile([128, DK, E], F32R)
            nc.sync.dma_start(rw, router_w[:, :].rearrange("(dk p) e -> p dk e", p=128))

            # combs[:, t, 0] = 1 (shared), combs[:, t, 1+e] = combine weight
            combs = big.tile([128, TT, NEXP], F32)
            for t in range(TT):
                psl = ps.tile([128, 512], F32, tag="l")
                pl = psl[:, :E]
                for dk in range(DK):
                    nc.tensor.matmul(
                        pl,
                        xt[:, dk, t * 128:(t + 1) * 128],
                        rw[:, dk],
                        start=(dk == 0),
                        stop=(dk == DK - 1),
                    )
                mx = small.tile([128, 1], F32, tag="mx")
                nc.vector.reduce_max(mx, pl, axis=AX)
                negmx = small.tile([128, 1], F32, tag="negmx")
                nc.vector.tensor_scalar_mul(negmx, mx, -1.0)
                ex = small.tile([128, E], F32, tag="ex")
                nc.scalar.activation(ex, pl, AF.Exp, bias=negmx, scale=1.0)
                sm = small.tile([128, 1], F32, tag="sm")
                nc.vector.reduce_sum(sm, ex, axis=AX)
                rs = small.tile([128, 1], F32, tag="rs")
                nc.vector.reciprocal(rs, sm)
                probs = small.tile([128, E], F32, tag="probs")
                nc.vector.tensor_scalar_mul(probs, ex, rs)
                # top-1
                m1 = small.tile([128, 1], F32, tag="m1")
                nc.vector.reduce_max(m1, probs, axis=AX)
                t1 = small.tile([128, E], F32, tag="t1")
                nc.vector.tensor_scalar(t1, probs, m1, None, ALU.is_ge)
                ptop = small.tile([128, E], F32, tag="ptop")
                nc.vector.tensor_mul(ptop, probs, t1)
                pm = small.tile([128, E], F32, tag="pm")
                nc.vector.tensor_sub(pm, probs, ptop)
                # top-2
                m2 = small.tile([128, 1], F32, tag="m2")
                nc.vector.reduce_max(m2, pm, axis=AX)
                t2 = small.tile([128, E], F32, tag="t2")
                nc.vector.tensor_scalar(t2, pm, m2, None, ALU.is_ge)
                p2 = small.tile([128, E], F32, tag="p2")
                nc.vector.tensor_mul(p2, pm, t2)
                nc.vector.tensor_add(combs[:, t, 1:], ptop, p2)
                nc.vector.memset(combs[:, t, 0:1], 1.0)

            # ---- output accumulator [128, TT, D] f32 (32KB/partition) ----
            out_acc = big.tile([128, TT, D], F32)

            # ---- expert loop: shared (j=0) + routed (j=1..8) ----
            hs = big.tile([128, HK, TPC], F32R)  # silu(G)*U, transposed [h, tok]
            for j in range(NEXP):
                # gate/up phase: 2 h-tiles (256 cols) per weight DMA
                for hb2 in range(HK // 2):
                    wg = wpool.tile([128, DK, 256], F32R, tag="wg")
                    wu = wpool.tile([128, DK, 256], F32R, tag="wu")
                    nc.sync.dma_start(
                        wg,
                        gate_all[j, :, hb2 * 256:(hb2 + 1) * 256].rearrange(
                            "(dk p) h -> p dk h", p=128
                        ),
                    )
                    nc.sync.dma_start(
                        wu,
                        up_all[j, :, hb2 * 256:(hb2 + 1) * 256].rearrange(
                            "(dk p) h -> p dk h", p=128
                        ),
                    )
                    for hh in range(2):
                        hb = hb2 * 2 + hh
                        for th in range(2):
                            tsl = slice(th * 512, (th + 1) * 512)
                            pg = ps.tile([128, 512], F32, tag="g")
                            pu = ps.tile([128, 512], F32, tag="u")
                            for dk in range(DK):
                                nc.tensor.matmul(
                                    pg,
                                    wg[:, dk, hh * 128:(hh + 1) * 128],
                                    xt[:, dk, tsl],
                                    start=(dk == 0),
                                    stop=(dk == DK - 1),
                                )
                            for dk in range(DK):
                                nc.tensor.matmul(
                                    pu,
                                    wu[:, dk, hh * 128:(hh + 1) * 128],
                                    xt[:, dk, tsl],
                                    start=(dk == 0),
                                    stop=(dk == DK - 1),
                                )
                            sg = small.tile([128, 512], F32, tag="sg")
                            nc.scalar.activation(sg, pg, AF.Silu)
                            nc.vector.tensor_mul(hs[:, hb, tsl], sg, pu)

                # down phase: d in quarters of 256
                for dq in range(4):
                    wd = wpool.tile([128, HK, 256], F32R, tag="wd")
                    nc.sync.dma_start(
                        wd,
                        down_all[j, :, dq * 256:(dq + 1) * 256].rearrange(
                            "(hk p) d -> p hk d", p=128
                        ),
                    )
                    for t in range(TT):
                        pz = ps.tile([128, 256], F32, tag="z")
                        for hk in range(HK):
                            nc.tensor.matmul(
                                pz,
                                hs[:, hk, t * 128:(t + 1) * 128],
                                wd[:, hk],
                                start=(hk == 0),
                                stop=(hk == HK - 1),
                            )
                        osl = out_acc[:, t, dq * 256:(dq + 1) * 256]
                        if j == 0:
                            nc.scalar.activation(osl, pz, AF.Copy)
                        else:
                            zt = small.tile([128, 256], F32, tag="zt")
                            nc.scalar.activation(
                                zt, pz, AF.Copy, scale=combs[:, t, j:j + 1]
                            )
                            nc.vector.tensor_add(osl, osl, zt)

            # ---- store ----
            nc.sync.dma_start(
                out[:, :].rearrange("(t p) d -> p t d", p=128), out_acc
            )

    _normalize_waits(nc)
    return nc


_built = {}


def _get_nc():
    if "nc" not in _built:
        _built["nc"] = build_nc()
    return _built["nc"]


def prepare_in_maps(x, router_w, shared_gate, shared_up, shared_down,
                    gate_w, up_w, down_w):
    xf = np.ascontiguousarray(np.asarray(x, np.float32).reshape(-1, D))
    gate_all = np.concatenate(
        [np.asarray(shared_gate, np.float32)[None], np.asarray(gate_w, np.float32)], 0
    )
    up_all = np.concatenate(
        [np.asarray(shared_up, np.float32)[None], np.asarray(up_w, np.float32)], 0
    )
    down_all = np.concatenate(
        [np.asarray(shared_down, np.float32)[None], np.asarray(down_w, np.float32)], 0
    )
    rw = np.ascontiguousarray(np.asarray(router_w, np.float32))
    in_maps = []
    for c in range(NCORES):
        xs = xf[c * TPC:(c + 1) * TPC]
        in_maps.append({
            "xT": np.ascontiguousarray(xs.T),
            "router_w": rw,
            "gate_all": gate_all,
            "up_all": up_all,
            "down_all": down_all,
        })
    return in_maps


def kernel(x, router_w, shared_gate, shared_up, shared_down,
           gate_w, up_w, down_w, top_k):
    assert int(top_k) == 2, "kernel hardcodes top-2 routing"
    x = np.asarray(x)
    assert x.size == NCORES * TPC * D, f"unexpected x shape {x.shape}"
    nc = _get_nc()
    in_maps = prepare_in_maps(
        x, router_w, shared_gate, shared_up, shared_down, gate_w, up_w, down_w
    )
    res = run_bass_kernel_spmd(nc, in_maps, list(range(NCORES)), trace=False)
    outs = [res.results[c]["out"] for c in range(NCORES)]
    full = np.concatenate(outs, axis=0)
    return full.reshape(np.asarray(x).shape).astype(np.float32)


# revision 3
# speedup vs baseline: 1.0222x; 1.0184x over previous
"""MoE (shared expert + 8 routed experts, top-2) on 8 TRN2 NeuronCores.

Strategy: data-parallel over tokens. Each core takes 1024 of the 8192 tokens
and computes the router (softmax + top-2 combine weights) and all 9 dense
SwiGLU expert MLPs (shared + 8 routed; routed scaled by per-token combine
weight, zero for non-routed tokens — mathematically identical to sparse
dispatch). Matmuls run in fp32r (full PE speed, ~1e-3 absolute accuracy).
Outputs concatenate across cores — no collectives needed.

Self-contained: hardcodes all shapes; relies only on the ambient concourse
environment (axon-tunneled TRN2 cores).
"""
import numpy as np

import bass_rust
import concourse.bass as bass
import concourse.mybir as mybir
import concourse.tile as tile
from concourse.bass_utils import run_bass_kernel_spmd

D = 1024          # d_model
H = 2048          # d_hidden per expert
E = 8             # routed experts
NCORES = 8
TPC = 1024        # tokens per core (8192 total / 8)
DK = D // 128     # 8 contraction tiles over d_model
HK = H // 128     # 16 contraction tiles over d_hidden
TT = TPC // 128   # 8 token tiles per core
NEXP = E + 1      # shared expert handled as expert 0 with combine weight 1

F32 = mybir.dt.float32
F32R = mybir.dt.float32r
AX = mybir.AxisListType.X
ALU = mybir.AluOpType
AF = mybir.ActivationFunctionType

_waitfix_ctr = [0]


def _normalize_waits(nc, max_waits=1):
    """This environment's walrus codegen accepts only one sync-wait slot per
    instruction; hoist extras onto standalone event-sem instructions."""
    for f in nc.m.functions:
        for blk in f.blocks:
            insts = list(blk.instructions)
            out = []
            changed = False
            for inst in insts:
                si = inst.sync_info
                waits = list(si.on_wait) if (si is not None and si.on_wait) else []
                if len(waits) > max_waits:
                    extra, keep = waits[:-max_waits], waits[-max_waits:]
                    for w in extra:
                        _waitfix_ctr[0] += 1
                        ev = mybir.InstEventSemaphore(
                            name=f"waitfix_{_waitfix_ctr[0]}", ins=[], outs=[]
                        )
                        ev.engine = inst.engine
                        ev.sync_info = bass_rust.SyncInfo(on_wait=[w], on_update=[])
                        out.append(ev)
                    si.on_wait = keep
                    changed = True
                out.append(inst)
            if changed:
                blk.instructions.clear()
                blk.instructions.extend(out)


def build_nc():
    nc = bass.Bass()

    # fp32r dram views of the (raw fp32) inputs; PE reads them at full rate.
    xT = nc.dram_tensor("xT", [D, TPC], F32R, kind="ExternalInput")
    router_w = nc.dram_tensor("router_w", [D, E], F32R, kind="ExternalInput")
    # gate/up: [9, D, H] (expert 0 = shared); down: [9, H, D]
    gate_all = nc.dram_tensor("gate_all", [NEXP, D, H], F32R, kind="ExternalInput")
    up_all = nc.dram_tensor("up_all", [NEXP, D, H], F32R, kind="ExternalInput")
    down_all = nc.dram_tensor("down_all", [NEXP, H, D], F32R, kind="ExternalInput")
    out = nc.dram_tensor("out", [TPC, D], F32, kind="ExternalOutput")

    with tile.TileContext(nc) as tc:
        with (
            tc.tile_pool(name="big", bufs=1) as big,       # persistent tiles
            tc.tile_pool(name="wpool", bufs=2) as wpool,   # streamed weight tiles
            tc.tile_pool(name="small", bufs=3) as small,   # elementwise temps
            tc.tile_pool(name="ps", bufs=2, space="PSUM") as ps,
        ):
            # ---- load x^T resident: [128, DK, TPC] (32KB/partition) ----
            xt = big.tile([128, DK, TPC], F32R)
            nc.sync.dma_start(xt, xT[:, :].rearrange("(dk p) t -> p dk t", p=128))

            # ---- router: probs, top-2 combine weights ----
            rw = big.tile([128, DK, E], F32R)
            nc.sync.dma_start(rw, router_w[:, :].rearrange("(dk p) e -> p dk e", p=128))

            # combs[:, t, 0] = 1 (shared), combs[:, t, 1+e] = combine weight
            combs = big.tile([128, TT, NEXP], F32)
            for t in range(TT):
                psl = ps.tile([128, 512], F32, tag="l")
                pl = psl[:, :E]
                for dk in range(DK):
                    nc.tensor.matmul(
                        pl,
                        xt[:, dk, t * 128:(t + 1) * 128],
                        rw[:, dk],
                        start=(dk == 0),
                        stop=(dk == DK - 1),
                    )
                mx = small.tile([128, 1], F32, tag="mx")
                nc.vector.reduce_max(mx, pl, axis=AX)
                negmx = small.tile([128, 1], F32, tag="negmx")
                nc.vector.tensor_scalar_mul(negmx, mx, -1.0)
                ex = small.tile([128, E], F32, tag="ex")
                nc.scalar.activation(ex, pl, AF.Exp, bias=negmx, scale=1.0)
                sm = small.tile([128, 1], F32, tag="sm")
                nc.vector.reduce_sum(sm, ex, axis=AX)
                rs = small.tile([128, 1], F32, tag="rs")
                nc.vector.reciprocal(rs, sm)
                probs = small.tile([128, E], F32, tag="probs")
                nc.vector.tensor_scalar_mul(probs, ex, rs)
                # top-1
                m1 = small.tile([128, 1], F32, tag="m1")
                nc.vector.reduce_max(m1, probs, axis=AX)
                t1 = small.tile([128, E], F32, tag="t1")
                nc.vector.tensor_scalar(t1, probs, m1, None, ALU.is_ge)
                ptop = small.tile([128, E], F32, tag="ptop")
                nc.vector.tensor_mul(ptop, probs, t1)
                pm = small.tile([128, E], F32, tag="pm")
                nc.vector.tensor_sub(pm, probs, ptop)
                # top-2
                m2 = small.tile([128, 1], F32, tag="m2")
                nc.vector.reduce_max(m2, pm, axis=AX)
                t2 = small.tile([128, E], F32, tag="t2")
                nc.vector.tensor_scalar(t2, pm, m2, None, ALU.is_ge)
                p2 = small.tile([128, E], F32, tag="p2")
                nc.vector.tensor_mul(p2, pm, t2)
                nc.vector.tensor_add(combs[:, t, 1:], ptop, p2)
                nc.vector.memset(combs[:, t, 0:1], 1.0)

            # ---- output accumulator [128, TT, D] f32 (32KB/partition) ----
            out_acc = big.tile([128, TT, D], F32)

            # ---- expert loop: shared (j=0) + routed (j=1..8) ----
            # Process each expert's hidden dim in halves of 1024 so the down
            # projection can use 512-wide moving operands (fewer, fuller
            # matmuls) while hs stays at 32KB/partition.
            HH = HK // 2  # 8 h-tiles per half
            hs = big.tile([128, HH, TPC], F32R)  # silu(G)*U for one h-half
            for j in range(NEXP):
                for hhalf in range(2):
                    h0 = hhalf * (H // 2)
                    # gate/up phase: 2 h-tiles (256 cols) per weight DMA
                    for hb2 in range(HH // 2):
                        c0 = h0 + hb2 * 256
                        wg = wpool.tile([128, DK, 256], F32R, tag="wg")
                        wu = wpool.tile([128, DK, 256], F32R, tag="wu")
                        nc.sync.dma_start(
                            wg,
                            gate_all[j, :, c0:c0 + 256].rearrange(
                                "(dk p) h -> p dk h", p=128
                            ),
                        )
                        nc.sync.dma_start(
                            wu,
                            up_all[j, :, c0:c0 + 256].rearrange(
                                "(dk p) h -> p dk h", p=128
                            ),
                        )
                        for hh in range(2):
                            hb = hb2 * 2 + hh  # local h-tile within the half
                            for th in range(2):
                                tsl = slice(th * 512, (th + 1) * 512)
                                pg = ps.tile([128, 512], F32, tag="g")
                                pu = ps.tile([128, 512], F32, tag="u")
                                for dk in range(DK):
                                    nc.tensor.matmul(
                                        pg,
                                        wg[:, dk, hh * 128:(hh + 1) * 128],
                                        xt[:, dk, tsl],
                                        start=(dk == 0),
                                        stop=(dk == DK - 1),
                                    )
                                for dk in range(DK):
                                    nc.tensor.matmul(
                                        pu,
                                        wu[:, dk, hh * 128:(hh + 1) * 128],
                                        xt[:, dk, tsl],
                                        start=(dk == 0),
                                        stop=(dk == DK - 1),
                                    )
                                sg = small.tile([128, 512], F32, tag="sg")
                                nc.scalar.activation(sg, pg, AF.Silu)
                                nc.vector.tensor_mul(hs[:, hb, tsl], sg, pu)

                    # down phase for this h-half: d in halves of 512
                    for dh in range(2):
                        wd = wpool.tile([128, HH, 512], F32R, tag="wd")
                        nc.sync.dma_start(
                            wd,
                            down_all[
                                j, h0:h0 + H // 2, dh * 512:(dh + 1) * 512
                            ].rearrange("(hk p) d -> p hk d", p=128),
                        )
                        for t in range(TT):
                            pz = ps.tile([128, 512], F32, tag="z")
                            for hk in range(HH):
                                nc.tensor.matmul(
                                    pz,
                                    hs[:, hk, t * 128:(t + 1) * 128],
                                    wd[:, hk],
                                    start=(hk == 0),
                                    stop=(hk == HH - 1),
                                )
                            osl = out_acc[:, t, dh * 512:(dh + 1) * 512]
                            if j == 0 and hhalf == 0:
                                nc.scalar.activation(osl, pz, AF.Copy)
                            else:
                                zt = small.tile([128, 512], F32, tag="zt")
                                if j == 0:
                                    nc.scalar.activation(zt, pz, AF.Copy)
                                else:
                                    nc.scalar.activation(
                                        zt, pz, AF.Copy,
                                        scale=combs[:, t, j:j + 1],
                                    )
                                nc.vector.tensor_add(osl, osl, zt)

            # ---- store ----
            nc.sync.dma_start(
                out[:, :].rearrange("(t p) d -> p t d", p=128), out_acc
            )

    _normalize_waits(nc)
    return nc


_built = {}


def _get_nc():
    if "nc" not in _built:
        _built["nc"] = build_nc()
    return _built["nc"]


def prepare_in_maps(x, router_w, shared_gate, shared_up, shared_down,
                    gate_w, up_w, down_w):
    xf = np.ascontiguousarray(np.asarray(x, np.float32).reshape(-1, D))
    gate_all = np.concatenate(
        [np.asarray(shared_gate, np.float32)[None], np.asarray(gate_w, np.float32)], 0
    )
    up_all = np.concatenate(
        [np.asarray(shared_up, np.float32)[None], np.asarray(up_w, np.float32)], 0
    )
    down_all = np.concatenate(
        [np.asarray(shared_down, np.float32)[None], np.asarray(down_w, np.float32)], 0
    )
    rw = np.ascontiguousarray(np.asarray(router_w, np.float32))
    in_maps = []
    for c in range(NCORES):
        xs = xf[c * TPC:(c + 1) * TPC]
        in_maps.append({
            "xT": np.ascontiguousarray(xs.T),
            "router_w": rw,
            "gate_all": gate_all,
            "up_all": up_all,
            "down_all": down_all,
        })
    return in_maps


def kernel(x, router_w, shared_gate, shared_up, shared_down,
           gate_w, up_w, down_w, top_k):
    assert int(top_k) == 2, "kernel hardcodes top-2 routing"
    x = np.asarray(x)
    assert x.size == NCORES * TPC * D, f"unexpected x shape {x.shape}"
    nc = _get_nc()
    in_maps = prepare_in_maps(
        x, router_w, shared_gate, shared_up, shared_down, gate_w, up_w, down_w
    )
    res = run_bass_kernel_spmd(nc, in_maps, list(range(NCORES)), trace=False)
    outs = [res.results[c]["out"] for c in range(NCORES)]
    full = np.concatenate(outs, axis=0)
    return full.reshape(np.asarray(x).shape).astype(np.float32)


# revision 5
# speedup vs baseline: 1.0245x; 1.0023x over previous
"""MoE (shared expert + 8 routed experts, top-2) on 8 TRN2 NeuronCores.

Strategy: data-parallel over tokens. Each core takes 1024 of the 8192 tokens
and computes the router (softmax + top-2 combine weights) and all 9 dense
SwiGLU expert MLPs (shared + 8 routed; routed scaled by per-token combine
weight, zero for non-routed tokens — mathematically identical to sparse
dispatch). Matmuls run in fp32r (full PE speed, ~1e-3 absolute accuracy).
Outputs concatenate across cores — no collectives needed.

Self-contained: hardcodes all shapes; relies only on the ambient concourse
environment (axon-tunneled TRN2 cores).
"""
import numpy as np

import bass_rust
import concourse.bass as bass
import concourse.mybir as mybir
import concourse.tile as tile
from concourse.bass_utils import run_bass_kernel_spmd

D = 1024          # d_model
H = 2048          # d_hidden per expert
E = 8             # routed experts
NCORES = 8
TPC = 1024        # tokens per core (8192 total / 8)
DK = D // 128     # 8 contraction tiles over d_model
HK = H // 128     # 16 contraction tiles over d_hidden
TT = TPC // 128   # 8 token tiles per core
NEXP = E + 1      # shared expert handled as expert 0 with combine weight 1

F32 = mybir.dt.float32
F32R = mybir.dt.float32r
AX = mybir.AxisListType.X
ALU = mybir.AluOpType
AF = mybir.ActivationFunctionType

_waitfix_ctr = [0]


def _normalize_waits(nc, max_waits=1):
    """This environment's walrus codegen accepts only one sync-wait slot per
    instruction; hoist extras onto standalone event-sem instructions."""
    for f in nc.m.functions:
        for blk in f.blocks:
            insts = list(blk.instructions)
            out = []
            changed = False
            for inst in insts:
                si = inst.sync_info
                waits = list(si.on_wait) if (si is not None and si.on_wait) else []
                if len(waits) > max_waits:
                    extra, keep = waits[:-max_waits], waits[-max_waits:]
                    for w in extra:
                        _waitfix_ctr[0] += 1
                        ev = mybir.InstEventSemaphore(
                            name=f"waitfix_{_waitfix_ctr[0]}", ins=[], outs=[]
                        )
                        ev.engine = inst.engine
                        ev.sync_info = bass_rust.SyncInfo(on_wait=[w], on_update=[])
                        out.append(ev)
                    si.on_wait = keep
                    changed = True
                out.append(inst)
            if changed:
                blk.instructions.clear()
                blk.instructions.extend(out)


def build_nc():
    nc = bass.Bass()

    # fp32r dram views of the (raw fp32) inputs; PE reads them at full rate.
    xT = nc.dram_tensor("xT", [D, TPC], F32R, kind="ExternalInput")
    router_w = nc.dram_tensor("router_w", [D, E], F32R, kind="ExternalInput")
    # gate/up: [9, D, H] (expert 0 = shared); down: [9, H, D]
    gate_all = nc.dram_tensor("gate_all", [NEXP, D, H], F32R, kind="ExternalInput")
    up_all = nc.dram_tensor("up_all", [NEXP, D, H], F32R, kind="ExternalInput")
    down_all = nc.dram_tensor("down_all", [NEXP, H, D], F32R, kind="ExternalInput")
    out = nc.dram_tensor("out", [TPC, D], F32, kind="ExternalOutput")

    with tile.TileContext(nc) as tc:
        with (
            tc.tile_pool(name="big", bufs=1) as big,       # persistent tiles
            tc.tile_pool(name="wpool", bufs=2) as wpool,   # streamed weight tiles
            tc.tile_pool(name="small", bufs=3) as small,   # elementwise temps
            tc.tile_pool(name="ps", bufs=2, space="PSUM") as ps,
        ):
            # ---- load x^T resident: [128, DK, TPC] (32KB/partition) ----
            # split per dk-chunk so the first matmuls start after 1/8 arrives
            xt = big.tile([128, DK, TPC], F32R)
            for dk in range(DK):
                nc.sync.dma_start(xt[:, dk], xT[dk * 128:(dk + 1) * 128, :])

            # ---- router: probs, top-2 combine weights ----
            rw = big.tile([128, DK, E], F32R)
            nc.sync.dma_start(rw, router_w[:, :].rearrange("(dk p) e -> p dk e", p=128))

            # combs[:, t, 0] = 1 (shared), combs[:, t, 1+e] = combine weight
            combs = big.tile([128, TT, NEXP], F32)
            for t in range(TT):
                psl = ps.tile([128, 512], F32, tag="l")
                pl = psl[:, :E]
                for dk in range(DK):
                    nc.tensor.matmul(
                        pl,
                        xt[:, dk, t * 128:(t + 1) * 128],
                        rw[:, dk],
                        start=(dk == 0),
                        stop=(dk == DK - 1),
                    )
                mx = small.tile([128, 1], F32, tag="mx")
                nc.vector.reduce_max(mx, pl, axis=AX)
                negmx = small.tile([128, 1], F32, tag="negmx")
                nc.vector.tensor_scalar_mul(negmx, mx, -1.0)
                ex = small.tile([128, E], F32, tag="ex")
                nc.scalar.activation(ex, pl, AF.Exp, bias=negmx, scale=1.0)
                sm = small.tile([128, 1], F32, tag="sm")
                nc.vector.reduce_sum(sm, ex, axis=AX)
                rs = small.tile([128, 1], F32, tag="rs")
                nc.vector.reciprocal(rs, sm)
                probs = small.tile([128, E], F32, tag="probs")
                nc.vector.tensor_scalar_mul(probs, ex, rs)
                # top-1
                m1 = small.tile([128, 1], F32, tag="m1")
                nc.vector.reduce_max(m1, probs, axis=AX)
                t1 = small.tile([128, E], F32, tag="t1")
                nc.vector.tensor_scalar(t1, probs, m1, None, ALU.is_ge)
                ptop = small.tile([128, E], F32, tag="ptop")
                nc.vector.tensor_mul(ptop, probs, t1)
                pm = small.tile([128, E], F32, tag="pm")
                nc.vector.tensor_sub(pm, probs, ptop)
                # top-2
                m2 = small.tile([128, 1], F32, tag="m2")
                nc.vector.reduce_max(m2, pm, axis=AX)
                t2 = small.tile([128, E], F32, tag="t2")
                nc.vector.tensor_scalar(t2, pm, m2, None, ALU.is_ge)
                p2 = small.tile([128, E], F32, tag="p2")
                nc.vector.tensor_mul(p2, pm, t2)
                nc.vector.tensor_add(combs[:, t, 1:], ptop, p2)
                nc.vector.memset(combs[:, t, 0:1], 1.0)

            # ---- output accumulator [128, TT, D] f32 (32KB/partition) ----
            out_acc = big.tile([128, TT, D], F32)

            # ---- expert loop: shared (j=0) + routed (j=1..8) ----
            # Process each expert's hidden dim in halves of 1024 so the down
            # projection can use 512-wide moving operands (fewer, fuller
            # matmuls) while hs stays at 32KB/partition.
            HH = HK // 2  # 8 h-tiles per half
            hs = big.tile([128, HH, TPC], F32R)  # silu(G)*U for one h-half
            for j in range(NEXP):
                for hhalf in range(2):
                    h0 = hhalf * (H // 2)
                    # gate/up phase: 2 h-tiles (256 cols) per weight DMA
                    for hb2 in range(HH // 2):
                        c0 = h0 + hb2 * 256
                        wg = wpool.tile([128, DK, 256], F32R, tag="wg")
                        wu = wpool.tile([128, DK, 256], F32R, tag="wu")
                        nc.sync.dma_start(
                            wg,
                            gate_all[j, :, c0:c0 + 256].rearrange(
                                "(dk p) h -> p dk h", p=128
                            ),
                        )
                        nc.sync.dma_start(
                            wu,
                            up_all[j, :, c0:c0 + 256].rearrange(
                                "(dk p) h -> p dk h", p=128
                            ),
                        )
                        for hh in range(2):
                            hb = hb2 * 2 + hh  # local h-tile within the half
                            for th in range(2):
                                tsl = slice(th * 512, (th + 1) * 512)
                                pg = ps.tile([128, 512], F32, tag="g")
                                pu = ps.tile([128, 512], F32, tag="u")
                                for dk in range(DK):
                                    nc.tensor.matmul(
                                        pg,
                                        wg[:, dk, hh * 128:(hh + 1) * 128],
                                        xt[:, dk, tsl],
                                        start=(dk == 0),
                                        stop=(dk == DK - 1),
                                    )
                                for dk in range(DK):
                                    nc.tensor.matmul(
                                        pu,
                                        wu[:, dk, hh * 128:(hh + 1) * 128],
                                        xt[:, dk, tsl],
                                        start=(dk == 0),
                                        stop=(dk == DK - 1),
                                    )
                                sg = small.tile([128, 512], F32, tag="sg")
                                nc.scalar.activation(sg, pg, AF.Silu)
                                nc.vector.tensor_mul(hs[:, hb, tsl], sg, pu)

                    # down phase for this h-half: d in halves of 512
                    for dh in range(2):
                        wd = wpool.tile([128, HH, 512], F32R, tag="wd")
                        nc.sync.dma_start(
                            wd,
                            down_all[
                                j, h0:h0 + H // 2, dh * 512:(dh + 1) * 512
                            ].rearrange("(hk p) d -> p hk d", p=128),
                        )
                        for t in range(TT):
                            pz = ps.tile([128, 512], F32, tag="z")
                            for hk in range(HH):
                                nc.tensor.matmul(
                                    pz,
                                    hs[:, hk, t * 128:(t + 1) * 128],
                                    wd[:, hk],
                                    start=(hk == 0),
                                    stop=(hk == HH - 1),
                                )
                            osl = out_acc[:, t, dh * 512:(dh + 1) * 512]
                            if j == 0 and hhalf == 0:
                                nc.scalar.activation(osl, pz, AF.Copy)
                            else:
                                zt = small.tile([128, 512], F32, tag="zt")
                                if j == 0:
                                    nc.scalar.activation(zt, pz, AF.Copy)
                                else:
                                    nc.scalar.activation(
                                        zt, pz, AF.Copy,
                                        scale=combs[:, t, j:j + 1],
                                    )
                                nc.vector.tensor_add(osl, osl, zt)
                            if j == NEXP - 1 and hhalf == 1:
                                # final contribution for this (t, dh) slice —
                                # stream it out while remaining slices compute
                                nc.sync.dma_start(
                                    out[
                                        t * 128:(t + 1) * 128,
                                        dh * 512:(dh + 1) * 512,
                                    ],
                                    osl,
                                )

    _normalize_waits(nc)
    return nc


_built = {}


def _get_nc():
    if "nc" not in _built:
        _built["nc"] = build_nc()
    return _built["nc"]


def prepare_in_maps(x, router_w, shared_gate, shared_up, shared_down,
                    gate_w, up_w, down_w):
    xf = np.ascontiguousarray(np.asarray(x, np.float32).reshape(-1, D))
    gate_all = np.concatenate(
        [np.asarray(shared_gate, np.float32)[None], np.asarray(gate_w, np.float32)], 0
    )
    up_all = np.concatenate(
        [np.asarray(shared_up, np.float32)[None], np.asarray(up_w, np.float32)], 0
    )
    down_all = np.concatenate(
        [np.asarray(shared_down, np.float32)[None], np.asarray(down_w, np.float32)], 0
    )
    rw = np.ascontiguousarray(np.asarray(router_w, np.float32))
    in_maps = []
    for c in range(NCORES):
        xs = xf[c * TPC:(c + 1) * TPC]
        in_maps.append({
            "xT": np.ascontiguousarray(xs.T),
            "router_w": rw,
            "gate_all": gate_all,
            "up_all": up_all,
            "down_all": down_all,
        })
    return in_maps


def kernel(x, router_w, shared_gate, shared_up, shared_down,
           gate_w, up_w, down_w, top_k):
    assert int(top_k) == 2, "kernel hardcodes top-2 routing"
    x = np.asarray(x)
    assert x.size == NCORES * TPC * D, f"unexpected x shape {x.shape}"
    nc = _get_nc()
    in_maps = prepare_in_maps(
        x, router_w, shared_gate, shared_up, shared_down, gate_w, up_w, down_w
    )
    res = run_bass_kernel_spmd(nc, in_maps, list(range(NCORES)), trace=False)
    outs = [res.results[c]["out"] for c in range(NCORES)]
    full = np.concatenate(outs, axis=0)
    return full.reshape(np.asarray(x).shape).astype(np.float32)
